# revision 9
# baseline (speedup 1.0000x reference)
"""Sliding-window multi-head attention (N=4, T=2048, D=1024, H=16, hd=64,
rotary over all 64 dims, window (128,128)) on 8 Trainium2 NeuronCores.

Sharding: data-parallel over (batch, sequence-half): core c handles batch
c//2, query tokens [h*1024, (h+1)*1024) with a 128-token KV halo on each
side (zero-padded at sequence edges, masked in softmax).

v2 design (vs v1 baseline at 338us):
  - all matmul operands bf16 (halves DMA bytes; kills the fp32r 4x
    small-free-dim penalty on banded score matmuls)
  - consolidated DMAs: one per 128-row chunk of each weight (~45 total vs
    265) so the serial HWDGE resource stops gating P1/P4
  - P1 q-projection in kt-outer waves so PE starts as soon as the first
    (x, w) chunk pair lands instead of after the full weight load
  - scores packed 3 psum banks per (qb, head) -> 3 exps instead of 6
  - AV accumulation uses per-region start flags (no zeroing matmul)
  - softmax denominators: ones-column rides in V; reciprocal row is
    broadcast across partitions with a stride-0 SBUF->SBUF DMA (no PE
    broadcast matmul, no ACT copy)
  - wout preloaded; output projection interleaved per query block
"""

import math

import ml_dtypes
import numpy as np

import bass_rust
import concourse.bass as bass
import concourse.mybir as mybir
import concourse.tile as tile
from concourse.bass_utils import run_bass_kernel_spmd
from concourse.vector_clock import ScopedClock

# ----------------------------------------------------------------------------
# Problem constants (hardcoded per the harness contract)
N, T, D = 4, 2048, 1024
H, HD = 16, 64
WINDOW = 128
ROPE_BASE = 10000.0
SCALE = 1.0 / math.sqrt(HD)

NCORES = 8
TQ = 1024             # query tokens per core
TE = TQ + 2 * WINDOW  # 1280 extended kv tokens per core
QB = 512              # query block
NQB = TQ // QB        # 2
KB = 128              # key block
NKB = (QB + 2 * WINDOW) // KB  # 6 key blocks per query block

VS = HD + 1   # per-head column stride in V (col 64 = ones)
VW = H * VS   # 1040

F32 = mybir.dt.float32
F32R = mybir.dt.float32r
BF16 = mybir.dt.bfloat16

# scores bank packing: (bank, bank_col, kb, q_off, width)
SC_PACK = [
    (0, 0,   0, 0,   128),
    (0, 128, 1, 0,   256),
    (0, 384, 2, 0,   128),
    (1, 0,   2, 128, 256),
    (1, 256, 3, 128, 256),
    (2, 0,   3, 384, 128),
    (2, 128, 4, 256, 256),
    (2, 384, 5, 384, 128),
]

# AV sub-matmuls: (bank, bank_col, kb, q_off, width)
# start=True ONLY on the first: it marks the whole 2KB psum bank pending-zero
# (ZERO_REGION_SIZE); later matmuls overwrite-then-accumulate per column via
# the per-element has_written bits. Ordered so every matmul's touched region
# is uniformly pending (first column write) or uniformly written (accumulate).
AV_PACK = [
    (0, 0,   0, 0,   128),
    (0, 128, 1, 0,   128),
    (0, 256, 1, 128, 128),
    (0, 384, 2, 0,   128),
    (1, 0,   2, 128, 128),
    (1, 128, 2, 256, 128),
    (1, 256, 3, 128, 256),
    (2, 0,   3, 384, 128),
    (2, 128, 4, 256, 256),
    (2, 384, 5, 384, 128),
]

_MAXW = 1  # this container's walrus accepts one sync wait per instruction


class SplitWaitTC(tile.TileContext):
    """TileContext that spreads multi-sem waits over NoOp carriers so every
    instruction carries at most one sync wait (codegen limit here)."""

    _waitnop_counter = 0

    def _split_waits(self, inst, commit):
        si = getattr(inst, "sync_info", None)
        if si is None:
            return
        waits = list(si.on_wait)
        if len(waits) <= _MAXW:
            return
        ups = list(si.on_update)
        head, keep = waits[:-_MAXW], waits[-_MAXW:]
        for w in head:
            nop = bass_rust.InstNoOp()
            nop.engine = inst.engine
            SplitWaitTC._waitnop_counter += 1
            nop.name = f"I-waitnop-{SplitWaitTC._waitnop_counter}"
            nop.bass_nofuse = True
            nop.sync_info = bass_rust.SyncInfo(on_wait=[w], on_update=[])
            commit(nop)
        inst.sync_info = bass_rust.SyncInfo(on_wait=keep, on_update=ups)

    def _commit_and_lower(self, inst, original_block, old_bb_map, bb_to_exit_bb):
        if isinstance(inst, mybir.Instruction) and not isinstance(
            inst, (tile.BassTileRelease,)
        ):
            self._split_waits(
                inst,
                lambda nop: super(SplitWaitTC, self)._commit_and_lower(
                    nop, original_block, old_bb_map, bb_to_exit_bb
                ),
            )
        return super()._commit_and_lower(inst, original_block, old_bb_map, bb_to_exit_bb)

    def _drain_and_barrier(self, tick_clock, wait_clock):
        probe = self.nc.sync.nop(nofuse=True)
        wait_clock.add_sem_waits(probe.ins, ScopedClock({None: tick_clock.global_clock}))
        si = probe.ins.sync_info
        waits = list(si.on_wait) if si is not None else []
        ups = list(si.on_update) if si is not None else []
        if len(waits) > _MAXW:
            probe.ins.sync_info = bass_rust.SyncInfo(on_wait=waits[:_MAXW], on_update=ups)
            rest = waits[_MAXW:]
            while rest:
                chunk, rest = rest[:_MAXW], rest[_MAXW:]
                n = self.nc.sync.nop(nofuse=True)
                n.ins.sync_info = bass_rust.SyncInfo(on_wait=chunk, on_update=[])
        self.nc.sync.drain()
        self.nc.all_engine_barrier()
        assert self.sems is not None
        popped = self.nc._tile_sem_poison_stack.pop()
        assert popped is self._sem_poison
        self.nc.clear_and_free_semaphores(list(self.sems.allocated().values()))
        self.nc.all_engine_barrier()


def _bcast_row(ap, n):
    """View of a single-partition row AP replicated n times via a stride-0
    free dim (the partition dim itself must keep a nonzero step)."""
    dims = [list(d) for d in list(ap.ap)]
    return bass.AP(ap.tensor, ap.offset, [dims[0], [0, n]] + dims[1:])


# ----------------------------------------------------------------------------
# Device program


def build_nc():
    nc = bass.Bass("TRN2", target_bir_lowering=False, debug=False, num_devices=NCORES)

    xt = nc.declare_dram_parameter("xt", [D, TE], BF16, isOutput=False)
    wq = nc.declare_dram_parameter("wq", [D, D], BF16, isOutput=False)
    wk = nc.declare_dram_parameter("wk", [D, D], BF16, isOutput=False)
    wv = nc.declare_dram_parameter("wv", [D, D], BF16, isOutput=False)
    wo = nc.declare_dram_parameter("wo", [D, D], BF16, isOutput=False)
    cq = nc.declare_dram_parameter("cq", [128, TQ], F32, isOutput=False)
    sq = nc.declare_dram_parameter("sq", [128, TQ], F32, isOutput=False)
    ck = nc.declare_dram_parameter("ck", [128, TE], F32, isOutput=False)
    sk = nc.declare_dram_parameter("sk", [128, TE], F32, isOutput=False)
    maskd = nc.declare_dram_parameter("mask", [128, NQB * 3 * QB], BF16, isOutput=False)
    permd = nc.declare_dram_parameter("perm", [128, 128], F32R, isOutput=False)
    yt = nc.declare_dram_parameter("yt", [D, TQ], F32, isOutput=True)

    AF = mybir.ActivationFunctionType

    with nc.allow_low_precision(reason="bf16 matmuls; fp32 accumulate"), SplitWaitTC(nc) as tc:
        with (
            tc.tile_pool(name="const", bufs=1) as constp,
            tc.tile_pool(name="persist", bufs=1) as persist,
        ):
            perm_t = constp.tile([128, 128], F32R, name="perm", tag="perm")
            nc.sync.dma_start(perm_t[:], permd[:])

            # persistent activations
            qT = [persist.tile([128, TQ], BF16, name=f"qT{i}", tag=f"qT{i}") for i in range(8)]
            kT = [persist.tile([128, TE], BF16, name=f"kT{i}", tag=f"kT{i}") for i in range(8)]
            vp = [persist.tile([128, VW], BF16, name=f"vp{i}", tag=f"vp{i}") for i in range(10)]
            aT = persist.tile([128, 8 * TQ], BF16, name="aT", tag="aT")

            # ---------------- P1: qkv projection + rope ----------------------
            with (
                tc.tile_pool(name="xw", bufs=1) as xw,
                tc.tile_pool(name="tabp", bufs=1) as tabp,
                tc.tile_pool(name="womask", bufs=1) as womask,
                tc.tile_pool(name="p1ps", bufs=6, space="PSUM") as p1ps,
                tc.tile_pool(name="swps", bufs=2, space="PSUM") as swps,
                tc.tile_pool(name="stage", bufs=2) as stage,
                tc.tile_pool(name="ropetmp", bufs=3) as ropetmp,
            ):
                xs = [xw.tile([128, TE], BF16, name=f"xs{i}", tag=f"xs{i}") for i in range(8)]
                wqs = [xw.tile([128, D], BF16, name=f"wqs{i}", tag=f"wqs{i}") for i in range(8)]
                wks = [xw.tile([128, D], BF16, name=f"wks{i}", tag=f"wks{i}") for i in range(8)]
                wvs = [xw.tile([128, D], BF16, name=f"wvs{i}", tag=f"wvs{i}") for i in range(8)]
                wos = [womask.tile([128, D], BF16, name=f"wos{i}", tag=f"wos{i}") for i in range(8)]
                # DMA order = issue order on SP: q weights + x first.
                for kt in range(8):
                    nc.sync.dma_start(wqs[kt][:], wq[kt * 128 : (kt + 1) * 128, :])
                    nc.sync.dma_start(xs[kt][:], xt[kt * 128 : (kt + 1) * 128, :])
                cq_t = tabp.tile([128, TQ], F32, name="cq", tag="cq")
                nc.sync.dma_start(cq_t[:], cq[:])
                sq_t = tabp.tile([128, TQ], F32, name="sq", tag="sq")
                nc.sync.dma_start(sq_t[:], sq[:])
                for kt in range(8):
                    nc.sync.dma_start(wks[kt][:], wk[kt * 128 : (kt + 1) * 128, :])
                ck_t = tabp.tile([128, TE], F32, name="ck", tag="ck")
                nc.sync.dma_start(ck_t[:], ck[:])
                sk_t = tabp.tile([128, TE], F32, name="sk", tag="sk")
                nc.sync.dma_start(sk_t[:], sk[:])
                for kt in range(8):
                    nc.sync.dma_start(wvs[kt][:], wv[kt * 128 : (kt + 1) * 128, :])
                for kt in range(8):
                    nc.sync.dma_start(wos[kt][:], wo[kt * 128 : (kt + 1) * 128, :])
                mask_t = womask.tile([128, NQB * 3 * QB], BF16, name="mask", tag="mask")
                nc.sync.dma_start(mask_t[:], maskd[:])

                def rope_block(dest_slice, ps, nt, ctab, stab, c0):
                    raw = stage.tile([128, 512], F32R, name="raw", tag="raw")
                    nc.scalar.copy(raw[:, :nt], ps[:, :nt])
                    psw = swps.tile([128, 512], F32, name="sw", tag="sw")
                    nc.tensor.matmul(
                        psw[:, :nt], perm_t[:], raw[:, :nt], start=True, stop=True
                    )
                    t1 = ropetmp.tile([128, 512], F32, name="t1", tag="t1")
                    nc.gpsimd.tensor_mul(t1[:, :nt], raw[:, :nt], ctab[:, c0 : c0 + nt])
                    t2 = ropetmp.tile([128, 512], F32, name="t2", tag="t2")
                    nc.vector.tensor_mul(t2[:, :nt], psw[:, :nt], stab[:, c0 : c0 + nt])
                    nc.vector.tensor_add(dest_slice, t1[:, :nt], t2[:, :nt])

                # q: kt-outer waves so PE starts on the first chunk pair
                qblocks = [(tb, m) for tb in range(2) for m in range(8)]
                for wave in (qblocks[0:6], qblocks[6:12], qblocks[12:16]):
                    pss = {}
                    for blk in wave:
                        pss[blk] = p1ps.tile([128, 512], F32, name="p1", tag="p1")
                    for kt in range(8):
                        for tb, m in wave:
                            nc.tensor.matmul(
                                pss[(tb, m)][:],
                                wqs[kt][:, m * 128 : (m + 1) * 128],
                                xs[kt][:, WINDOW + tb * 512 : WINDOW + (tb + 1) * 512],
                                start=(kt == 0),
                                stop=(kt == 7),
                            )
                    for tb, m in wave:
                        rope_block(
                            qT[m][:, tb * 512 : (tb + 1) * 512],
                            pss[(tb, m)], 512, cq_t, sq_t, tb * 512,
                        )

                # k: all chunks resident by now; kt-inner
                for m in range(8):
                    for tb in range(3):
                        t0, nt = tb * 512, min(512, TE - tb * 512)
                        ps = p1ps.tile([128, 512], F32, name="p1", tag="p1")
                        for kt in range(8):
                            nc.tensor.matmul(
                                ps[:, :nt],
                                wks[kt][:, m * 128 : (m + 1) * 128],
                                xs[kt][:, t0 : t0 + nt],
                                start=(kt == 0),
                                stop=(kt == 7),
                            )
                        rope_block(kT[m][:, t0 : t0 + nt], ps, nt, ck_t, sk_t, t0)

                # v: token-major [tok, head-strided feat]
                for tt in range(10):
                    for fb in range(2):
                        ps = p1ps.tile([128, 512], F32, name="p1", tag="p1")
                        for kt in range(8):
                            nc.tensor.matmul(
                                ps[:],
                                xs[kt][:, tt * 128 : (tt + 1) * 128],
                                wvs[kt][:, fb * 512 : (fb + 1) * 512],
                                start=(kt == 0),
                                stop=(kt == 7),
                            )
                        dst = vp[tt][:].rearrange("p (h s) -> p h s", s=VS)[
                            :, fb * 8 : (fb + 1) * 8, :HD
                        ]
                        nc.scalar.copy(dst, ps[:].rearrange("p (h s) -> p h s", s=HD))
                    onescols = vp[tt][:].rearrange("p (h s) -> p h s", s=VS)[:, :, HD:]
                    nc.gpsimd.memset(onescols, 1.0)

            # ---------------- P3: banded attention + P4 interleaved ----------
            with (
                tc.tile_pool(name="sps", bufs=3, space="PSUM") as sps,
                tc.tile_pool(name="avps", bufs=4, space="PSUM") as avps,
                tc.tile_pool(name="yps", bufs=1, space="PSUM") as yps,
                tc.tile_pool(name="probs", bufs=6) as probsp,
                tc.tile_pool(name="rcp", bufs=4) as rcp,
                tc.tile_pool(name="bcp", bufs=4) as bcp,
                tc.tile_pool(name="anp", bufs=3) as anp,
                tc.tile_pool(name="yst", bufs=2) as yst,
            ):
                for qb in range(NQB):
                    for h in range(H):
                        ft = h // 2
                        p0 = (h % 2) * 64
                        # scores into 3 packed psum banks
                        banks = [
                            sps.tile([128, QB], F32, name="s", tag="s")
                            for _ in range(3)
                        ]
                        for bank, bcol, kb, qoff, w in SC_PACK:
                            kv0 = (qb * 4 + kb) * 128
                            nc.tensor.matmul(
                                banks[bank][:, bcol : bcol + w],
                                kT[ft][p0 : p0 + 64, kv0 : kv0 + KB],
                                qT[ft][p0 : p0 + 64, qb * QB + qoff : qb * QB + qoff + w],
                                start=True,
                                stop=True,
                            )
                        prs = []
                        for b in range(3):
                            pr = probsp.tile([128, QB], BF16, name="pr", tag="pr")
                            nc.scalar.activation(pr[:], banks[b][:], AF.Exp, scale=SCALE)
                            mcol = (qb * 3 + b) * QB
                            if b == 2:  # offload one of three mask muls to Pool
                                nc.gpsimd.tensor_mul(
                                    pr[:], pr[:], mask_t[:, mcol : mcol + QB]
                                )
                            else:
                                nc.vector.tensor_mul(
                                    pr[:], pr[:], mask_t[:, mcol : mcol + QB]
                                )
                            prs.append(pr)
                        # AV with per-region start flags; ones column rides
                        # along as output row 64 = masked prob sums
                        psX = avps.tile([128, QB], F32, name="av", tag="av")
                        nav = len(AV_PACK)
                        for i, (bank, bcol, kb, qoff, w) in enumerate(AV_PACK):
                            vt = qb * 4 + kb
                            nc.tensor.matmul(
                                psX[: VS, qoff : qoff + w],
                                vp[vt][:, h * VS : h * VS + VS],
                                prs[bank][:, bcol : bcol + w],
                                start=(i == 0),
                                stop=(i == nav - 1),
                                skip_group_check=True,
                            )
                        # normalize: reciprocal row -> stride-0 DMA broadcast
                        rc = rcp.tile([VS, QB], F32, name="rc", tag="rc")
                        nc.vector.reciprocal(rc[HD : HD + 1, :], psX[HD : HD + 1, :])
                        bc = bcp.tile([64, QB], F32, name="bc", tag="bc")
                        nc.sync.dma_start(bc[:, :], _bcast_row(rc[HD : HD + 1, :], 64))
                        if p0 == 0:
                            nc.vector.tensor_mul(
                                aT[0:HD, ft * TQ + qb * QB : ft * TQ + (qb + 1) * QB],
                                psX[0:HD, :],
                                bc[:],
                            )
                        else:
                            an = anp.tile([64, QB], BF16, name="an", tag="an")
                            nc.vector.tensor_mul(an[:], psX[0:HD, :], bc[:])
                            # partition-shift DMA into the odd-head rows of aT
                            nc.sync.dma_start(
                                aT[HD:128, ft * TQ + qb * QB : ft * TQ + (qb + 1) * QB],
                                an[:],
                            )
                    # P4 for this query block
                    for mo in range(8):
                        ps = yps.tile([128, QB], F32, name="y", tag="y")
                        for kf in range(8):
                            nc.tensor.matmul(
                                ps[:],
                                wos[kf][:, mo * 128 : (mo + 1) * 128],
                                aT[:, kf * TQ + qb * QB : kf * TQ + (qb + 1) * QB],
                                start=(kf == 0),
                                stop=(kf == 7),
                            )
                        ys = yst.tile([128, QB], F32, name="ys", tag="ys")
                        nc.scalar.copy(ys[:], ps[:])
                        nc.sync.dma_start(
                            yt[mo * 128 : (mo + 1) * 128, qb * QB : (qb + 1) * QB],
                            ys[:],
                        )

    return nc


# ----------------------------------------------------------------------------
# Host-side shard preparation


def _rope_tables(pos):
    """[128, len(pos)] cos and signed-sin tables for the 2-head tile layout."""
    inv_freq = 1.0 / (ROPE_BASE ** (np.arange(0, HD, 2, dtype=np.float32) / HD))  # [32]
    freqs = np.outer(pos.astype(np.float32), inv_freq)  # [T, 32]
    c32 = np.cos(freqs).astype(np.float32).T  # [32, T]
    s32 = np.sin(freqs).astype(np.float32).T
    ctab = np.tile(c32, (4, 1))  # rows r use freq r%32
    sgn = np.repeat(np.array([-1.0, 1.0, -1.0, 1.0], dtype=np.float32), 32)
    stab = np.tile(s32, (4, 1)) * sgn[:, None]
    return np.ascontiguousarray(ctab), np.ascontiguousarray(stab)


def _perm_matrix():
    p = np.zeros((128, 128), dtype=np.float32)
    for i in range(128):
        j = i + 32 if (i // 32) % 2 == 0 else i - 32
        p[i, j] = 1.0
    return p


_BF = ml_dtypes.bfloat16


def _shared_inputs(Wqkv, Wout):
    Wqkv = np.asarray(Wqkv, dtype=np.float32)
    return {
        "wq": np.ascontiguousarray(Wqkv[:, 0:D]).astype(_BF),
        "wk": np.ascontiguousarray(Wqkv[:, D : 2 * D]).astype(_BF),
        "wv": np.ascontiguousarray(Wqkv[:, 2 * D : 3 * D]).astype(_BF),
        "wo": np.ascontiguousarray(np.asarray(Wout, dtype=np.float32)).astype(_BF),
        "perm": _perm_matrix(),
    }


def _core_inputs(x, shared, core):
    n, half = core // 2, core % 2
    q0 = half * TQ            # first query token (global)
    e0 = q0 - WINDOW          # first ext kv token (global, may be negative)

    x_ext = np.zeros((TE, D), dtype=np.float32)
    lo, hi = max(e0, 0), min(e0 + TE, T)
    x_ext[lo - e0 : hi - e0] = x[n, lo:hi]
    xt = np.ascontiguousarray(x_ext.T).astype(_BF)

    pos_q = np.arange(q0, q0 + TQ)
    pos_k = np.clip(np.arange(e0, e0 + TE), 0, T - 1)
    cqt, sqt = _rope_tables(pos_q)
    ckt, skt = _rope_tables(pos_k)

    # mask [128 kt, NQB*3*QB] in the packed 3-bank layout
    mask = np.zeros((128, NQB * 3 * QB), dtype=np.float32)
    for qb in range(NQB):
        for bank, bcol, kb, qoff, w in SC_PACK:
            jj = e0 + qb * QB + kb * KB + np.arange(KB)       # global key index
            ii = q0 + qb * QB + qoff + np.arange(w)           # global query index
            valid = (
                (np.abs(jj[:, None] - ii[None, :]) <= WINDOW)
                & (jj[:, None] >= 0)
                & (jj[:, None] < T)
            )
            c0 = (qb * 3 + bank) * QB + bcol
            mask[:, c0 : c0 + w] = valid
    out = dict(shared)
    out.update({
        "xt": xt,
        "cq": cqt,
        "sq": sqt,
        "ck": ckt,
        "sk": skt,
        "mask": mask.astype(_BF),
    })
    return out


_NC_CACHE = {}


def _get_nc():
    if "nc" not in _NC_CACHE:
        _NC_CACHE["nc"] = build_nc()
    return _NC_CACHE["nc"]


def kernel(x, Wqkv, Wout, bout, _trace=False, _trace_kwargs=None):
    x = np.asarray(x, dtype=np.float32)
    shared = _shared_inputs(Wqkv, Wout)
    in_maps = [_core_inputs(x, shared, c) for c in range(NCORES)]
    nc = _get_nc()
    kw = {}
    if _trace:
        kw = {"trace": True, "trace_kwargs": _trace_kwargs or {}}
    res = run_bass_kernel_spmd(nc, in_maps, core_ids=list(range(NCORES)), **kw)
    out = np.empty((N, T, D), dtype=np.float32)
    for c in range(NCORES):
        n, half = c // 2, c % 2
        out[n, half * TQ : (half + 1) * TQ] = res.results[c]["yt"].T
    out += np.asarray(bout, dtype=np.float32)[None, None, :]
    kernel._last_results = res
    return out


# revision 19
# speedup vs baseline: 1.0677x; 1.0677x over previous
"""Sliding-window multi-head attention (N=4, T=2048, D=1024, H=16, hd=64,
rotary over all 64 dims, window (128,128)) on 8 Trainium2 NeuronCores.

Sharding: data-parallel over (batch, sequence-half): core c handles batch
c//2, query tokens [h*1024, (h+1)*1024) with a 128-token KV halo on each
side (zero-padded at sequence edges, masked in softmax).

v2 design (vs v1 baseline at 338us):
  - all matmul operands bf16 (halves DMA bytes; kills the fp32r 4x
    small-free-dim penalty on banded score matmuls)
  - consolidated DMAs: one per 128-row chunk of each weight (~45 total vs
    265) so the serial HWDGE resource stops gating P1/P4
  - P1 q-projection in kt-outer waves so PE starts as soon as the first
    (x, w) chunk pair lands instead of after the full weight load
  - scores packed 3 psum banks per (qb, head) -> 3 exps instead of 6
  - AV accumulation uses per-region start flags (no zeroing matmul)
  - softmax denominators: ones-column rides in V; reciprocal row is
    broadcast across partitions with a stride-0 SBUF->SBUF DMA (no PE
    broadcast matmul, no ACT copy)
  - wout preloaded; output projection interleaved per query block
"""

import math

import ml_dtypes
import numpy as np

import bass_rust
import concourse.bass as bass
import concourse.mybir as mybir
import concourse.tile as tile
from concourse.bass_utils import run_bass_kernel_spmd
from concourse.vector_clock import ScopedClock

# ----------------------------------------------------------------------------
# Problem constants (hardcoded per the harness contract)
N, T, D = 4, 2048, 1024
H, HD = 16, 64
WINDOW = 128
ROPE_BASE = 10000.0
SCALE = 1.0 / math.sqrt(HD)

NCORES = 8
TQ = 1024             # query tokens per core
TE = TQ + 2 * WINDOW  # 1280 extended kv tokens per core
QB = 512              # query block
NQB = TQ // QB        # 2
KB = 128              # key block
NKB = (QB + 2 * WINDOW) // KB  # 6 key blocks per query block

VS = HD + 1   # per-head column stride in V (col 64 = ones)
VW = H * VS   # 1040

F32 = mybir.dt.float32
F32R = mybir.dt.float32r
BF16 = mybir.dt.bfloat16

# scores bank packing: (bank, bank_col, kb, q_off, width)
SC_PACK = [
    (0, 0,   0, 0,   128),
    (0, 128, 1, 0,   256),
    (0, 384, 2, 0,   128),
    (1, 0,   2, 128, 256),
    (1, 256, 3, 128, 256),
    (2, 0,   3, 384, 128),
    (2, 128, 4, 256, 256),
    (2, 384, 5, 384, 128),
]

# AV sub-matmuls: (bank, bank_col, kb, q_off, width)
# start=True ONLY on the first: it marks the whole 2KB psum bank pending-zero
# (ZERO_REGION_SIZE); later matmuls overwrite-then-accumulate per column via
# the per-element has_written bits. Ordered so every matmul's touched region
# is uniformly pending (first column write) or uniformly written (accumulate).
AV_PACK = [
    (0, 0,   0, 0,   128),
    (0, 128, 1, 0,   128),
    (0, 256, 1, 128, 128),
    (0, 384, 2, 0,   128),
    (1, 0,   2, 128, 128),
    (1, 128, 2, 256, 128),
    (1, 256, 3, 128, 256),
    (2, 0,   3, 384, 128),
    (2, 128, 4, 256, 256),
    (2, 384, 5, 384, 128),
]

_MAXW = 1  # this container's walrus accepts one sync wait per instruction


class SplitWaitTC(tile.TileContext):
    """TileContext that spreads multi-sem waits over NoOp carriers so every
    instruction carries at most one sync wait (codegen limit here)."""

    _waitnop_counter = 0

    def _split_waits(self, inst, commit):
        si = getattr(inst, "sync_info", None)
        if si is None:
            return
        waits = list(si.on_wait)
        if len(waits) <= _MAXW:
            return
        ups = list(si.on_update)
        head, keep = waits[:-_MAXW], waits[-_MAXW:]
        for w in head:
            nop = bass_rust.InstNoOp()
            nop.engine = inst.engine
            SplitWaitTC._waitnop_counter += 1
            nop.name = f"I-waitnop-{SplitWaitTC._waitnop_counter}"
            nop.bass_nofuse = True
            nop.sync_info = bass_rust.SyncInfo(on_wait=[w], on_update=[])
            commit(nop)
        inst.sync_info = bass_rust.SyncInfo(on_wait=keep, on_update=ups)

    def _commit_and_lower(self, inst, original_block, old_bb_map, bb_to_exit_bb):
        if isinstance(inst, mybir.Instruction) and not isinstance(
            inst, (tile.BassTileRelease,)
        ):
            self._split_waits(
                inst,
                lambda nop: super(SplitWaitTC, self)._commit_and_lower(
                    nop, original_block, old_bb_map, bb_to_exit_bb
                ),
            )
        return super()._commit_and_lower(inst, original_block, old_bb_map, bb_to_exit_bb)

    def _drain_and_barrier(self, tick_clock, wait_clock):
        probe = self.nc.sync.nop(nofuse=True)
        wait_clock.add_sem_waits(probe.ins, ScopedClock({None: tick_clock.global_clock}))
        si = probe.ins.sync_info
        waits = list(si.on_wait) if si is not None else []
        ups = list(si.on_update) if si is not None else []
        if len(waits) > _MAXW:
            probe.ins.sync_info = bass_rust.SyncInfo(on_wait=waits[:_MAXW], on_update=ups)
            rest = waits[_MAXW:]
            while rest:
                chunk, rest = rest[:_MAXW], rest[_MAXW:]
                n = self.nc.sync.nop(nofuse=True)
                n.ins.sync_info = bass_rust.SyncInfo(on_wait=chunk, on_update=[])
        self.nc.sync.drain()
        self.nc.all_engine_barrier()
        assert self.sems is not None
        popped = self.nc._tile_sem_poison_stack.pop()
        assert popped is self._sem_poison
        self.nc.clear_and_free_semaphores(list(self.sems.allocated().values()))
        self.nc.all_engine_barrier()


def _bcast_row(ap, n):
    """View of a single-partition row AP replicated n times via a stride-0
    free dim (the partition dim itself must keep a nonzero step)."""
    dims = [list(d) for d in list(ap.ap)]
    return bass.AP(ap.tensor, ap.offset, [dims[0], [0, n]] + dims[1:])


# ----------------------------------------------------------------------------
# Device program


def build_nc():
    nc = bass.Bass("TRN2", target_bir_lowering=False, debug=False, num_devices=NCORES)

    xt = nc.declare_dram_parameter("xt", [D, TE], BF16, isOutput=False)
    wq = nc.declare_dram_parameter("wq", [D, D], BF16, isOutput=False)
    wk = nc.declare_dram_parameter("wk", [D, D], BF16, isOutput=False)
    wv = nc.declare_dram_parameter("wv", [D, D], BF16, isOutput=False)
    wo = nc.declare_dram_parameter("wo", [D, D], BF16, isOutput=False)
    cq = nc.declare_dram_parameter("cq", [128, TQ], BF16, isOutput=False)
    sq = nc.declare_dram_parameter("sq", [128, TQ], F32, isOutput=False)
    ck = nc.declare_dram_parameter("ck", [128, TE], BF16, isOutput=False)
    sk = nc.declare_dram_parameter("sk", [128, TE], F32, isOutput=False)
    maskd = nc.declare_dram_parameter("mask", [128, NQB * 3 * QB], BF16, isOutput=False)
    permd = nc.declare_dram_parameter("perm", [128, 128], BF16, isOutput=False)
    yt = nc.declare_dram_parameter("yt", [D, TQ], F32, isOutput=True)

    AF = mybir.ActivationFunctionType

    with nc.allow_low_precision(reason="bf16 matmuls; fp32 accumulate"), SplitWaitTC(nc) as tc:
        with (
            tc.tile_pool(name="const", bufs=1) as constp,
            tc.tile_pool(name="persist", bufs=1) as persist,
        ):
            perm_t = constp.tile([128, 128], BF16, name="perm", tag="perm")
            nc.sync.dma_start(perm_t[:], permd[:])

            # persistent activations
            qT = [persist.tile([128, TQ], BF16, name=f"qT{i}", tag=f"qT{i}") for i in range(8)]
            kT = [persist.tile([128, TE], BF16, name=f"kT{i}", tag=f"kT{i}") for i in range(8)]
            vp = [persist.tile([128, VW], BF16, name=f"vp{i}", tag=f"vp{i}") for i in range(10)]
            aT = persist.tile([128, 8 * TQ], BF16, name="aT", tag="aT")

            # shared attention-side SBUF pools (span all phases)
            with (
                tc.tile_pool(name="maskp", bufs=1) as maskp,
                tc.tile_pool(name="probs", bufs=7) as probsp,
                tc.tile_pool(name="rcp", bufs=3) as rcp,
                tc.tile_pool(name="bcp", bufs=3) as bcp,
                tc.tile_pool(name="anp", bufs=2) as anp,
                tc.tile_pool(name="yst", bufs=2) as yst,
            ):
                mask_t = maskp.tile([128, NQB * 3 * QB], BF16, name="mask", tag="mask")

                def attn_head(qb, h, spool, apool):
                    ft = h // 2
                    p0 = (h % 2) * 64
                    banks = [
                        spool.tile([128, QB], F32, name="s", tag="s") for _ in range(3)
                    ]
                    for bank, bcol, kb, qoff, w in SC_PACK:
                        kv0 = (qb * 4 + kb) * 128
                        nc.tensor.matmul(
                            banks[bank][:, bcol : bcol + w],
                            kT[ft][p0 : p0 + 64, kv0 : kv0 + KB],
                            qT[ft][p0 : p0 + 64, qb * QB + qoff : qb * QB + qoff + w],
                            start=True,
                            stop=True,
                        )
                    prs = []
                    for b in range(3):
                        pr = probsp.tile([128, QB], BF16, name="pr", tag="pr")
                        nc.scalar.activation(pr[:], banks[b][:], AF.Exp, scale=SCALE)
                        mcol = (qb * 3 + b) * QB
                        if b == 2 or (b == 1 and h % 2 == 1):
                            # offload 1.5 of three mask muls to Pool
                            nc.gpsimd.tensor_mul(pr[:], pr[:], mask_t[:, mcol : mcol + QB])
                        else:
                            nc.vector.tensor_mul(pr[:], pr[:], mask_t[:, mcol : mcol + QB])
                        prs.append(pr)
                    # AV: start only on the first matmul (marks whole bank
                    # pending-zero); later ones overwrite-then-accumulate per
                    # column. The V ones column lands in output row 64 =
                    # masked prob sums.
                    psX = apool.tile([128, QB], F32, name="av", tag="av")
                    nav = len(AV_PACK)
                    for i, (bank, bcol, kb, qoff, w) in enumerate(AV_PACK):
                        vt = qb * 4 + kb
                        nc.tensor.matmul(
                            psX[: VS, qoff : qoff + w],
                            vp[vt][:, h * VS : h * VS + VS],
                            prs[bank][:, bcol : bcol + w],
                            start=(i == 0),
                            stop=(i == nav - 1),
                            skip_group_check=True,
                        )
                    # normalize: reciprocal row -> stride-0 DMA broadcast
                    rc = rcp.tile([VS, QB], F32, name="rc", tag="rc")
                    nc.vector.reciprocal(rc[HD : HD + 1, :], psX[HD : HD + 1, :])
                    bc = bcp.tile([64, QB], F32, name="bc", tag="bc")
                    nc.sync.dma_start(bc[:, :], _bcast_row(rc[HD : HD + 1, :], 64))
                    if p0 == 0:
                        nc.vector.tensor_mul(
                            aT[0:HD, ft * TQ + qb * QB : ft * TQ + (qb + 1) * QB],
                            psX[0:HD, :],
                            bc[:],
                        )
                    else:
                        an = anp.tile([64, QB], BF16, name="an", tag="an")
                        nc.vector.tensor_mul(an[:], psX[0:HD, :], bc[:])
                        # partition-shift DMA into the odd-head rows of aT
                        nc.sync.dma_start(
                            aT[HD:128, ft * TQ + qb * QB : ft * TQ + (qb + 1) * QB],
                            an[:],
                        )

                # ---------------- P1 + attention qb0 -------------------------
                with (
                    tc.tile_pool(name="p1data", bufs=1) as p1data,
                    tc.tile_pool(name="stage", bufs=2) as stage,
                    tc.tile_pool(name="ropetmp", bufs=2) as ropetmp,
                ):
                    xs = [p1data.tile([128, TE], BF16, name=f"xs{i}", tag=f"xs{i}") for i in range(8)]
                    wqs = [p1data.tile([128, D], BF16, name=f"wqs{i}", tag=f"wqs{i}") for i in range(8)]
                    wks = [p1data.tile([128, D], BF16, name=f"wks{i}", tag=f"wks{i}") for i in range(8)]
                    wvs = [p1data.tile([128, D], BF16, name=f"wvs{i}", tag=f"wvs{i}") for i in range(8)]
                    # DMA order = issue order on SP: q weights + x first.
                    for kt in range(8):
                        nc.sync.dma_start(wqs[kt][:], wq[kt * 128 : (kt + 1) * 128, :])
                        nc.sync.dma_start(xs[kt][:], xt[kt * 128 : (kt + 1) * 128, :])
                    cq_t = p1data.tile([128, TQ], BF16, name="cq", tag="cq")
                    nc.sync.dma_start(cq_t[:], cq[:])
                    sq_t = p1data.tile([128, TQ], F32, name="sq", tag="sq")
                    nc.sync.dma_start(sq_t[:], sq[:])
                    for kt in range(8):
                        nc.sync.dma_start(wks[kt][:], wk[kt * 128 : (kt + 1) * 128, :])
                    ck_t = p1data.tile([128, TE], BF16, name="ck", tag="ck")
                    nc.sync.dma_start(ck_t[:], ck[:])
                    sk_t = p1data.tile([128, TE], F32, name="sk", tag="sk")
                    nc.sync.dma_start(sk_t[:], sk[:])
                    for kt in range(8):
                        nc.sync.dma_start(wvs[kt][:], wv[kt * 128 : (kt + 1) * 128, :])
                    nc.sync.dma_start(mask_t[:], maskd[:])

                    def rope_block(dest_slice, ps, nt, ctab, stab, c0, swpool, swtag):
                        raw = stage.tile([128, 512], BF16, name="raw", tag="raw")
                        nc.scalar.copy(raw[:, :nt], ps[:, :nt])
                        psw = swpool.tile([128, 512], F32, name="sw", tag=swtag)
                        nc.tensor.matmul(
                            psw[:, :nt], perm_t[:], raw[:, :nt], start=True, stop=True
                        )
                        t1 = ropetmp.tile([128, 512], BF16, name="t1", tag="t1")
                        nc.gpsimd.tensor_mul(t1[:, :nt], raw[:, :nt], ctab[:, c0 : c0 + nt])
                        t2 = ropetmp.tile([128, 512], BF16, name="t2", tag="t2")
                        nc.vector.tensor_mul(t2[:, :nt], psw[:, :nt], stab[:, c0 : c0 + nt])
                        nc.vector.tensor_add(dest_slice, t1[:, :nt], t2[:, :nt])

                    def proj_block(ps, nt, wtiles, mcol, xcol):
                        for kt in range(8):
                            nc.tensor.matmul(
                                ps[:, :nt],
                                wtiles[kt][:, mcol : mcol + 128],
                                xs[kt][:, xcol : xcol + nt],
                                start=(kt == 0),
                                stop=(kt == 7),
                            )

                    def v_block(tt, pool, tag):
                        for fb in range(2):
                            ps = pool.tile([128, 512], F32, name="p1", tag=tag)
                            for kt in range(8):
                                nc.tensor.matmul(
                                    ps[:],
                                    xs[kt][:, tt * 128 : (tt + 1) * 128],
                                    wvs[kt][:, fb * 512 : (fb + 1) * 512],
                                    start=(kt == 0),
                                    stop=(kt == 7),
                                )
                            dst = vp[tt][:].rearrange("p (h s) -> p h s", s=VS)[
                                :, fb * 8 : (fb + 1) * 8, :HD
                            ]
                            nc.scalar.copy(dst, ps[:].rearrange("p (h s) -> p h s", s=HD))
                        onescols = vp[tt][:].rearrange("p (h s) -> p h s", s=VS)[:, :, HD:]
                        nc.gpsimd.memset(onescols, 1.0)

                    # scope 1: q tb0 (kt-outer waves for early PE start),
                    # k tb0/tb1, v tt0..5 — everything attention qb0 needs
                    with (
                        tc.tile_pool(name="p1ps", bufs=6, space="PSUM") as p1ps,
                        tc.tile_pool(name="swps", bufs=2, space="PSUM") as swps,
                    ):
                        for wave in (list(range(6)), list(range(6, 8))):
                            pss = {}
                            for m in wave:
                                pss[m] = p1ps.tile([128, 512], F32, name="p1", tag="p1")
                            for kt in range(8):
                                for m in wave:
                                    nc.tensor.matmul(
                                        pss[m][:],
                                        wqs[kt][:, m * 128 : (m + 1) * 128],
                                        xs[kt][:, WINDOW : WINDOW + 512],
                                        start=(kt == 0),
                                        stop=(kt == 7),
                                    )
                            for m in wave:
                                rope_block(qT[m][:, 0:512], pss[m], 512, cq_t, sq_t, 0,
                                           swps, "sw")
                        for m in range(8):
                            for tb in range(2):
                                t0 = tb * 512
                                ps = p1ps.tile([128, 512], F32, name="p1", tag="p1")
                                proj_block(ps, 512, wks, m * 128, t0)
                                rope_block(kT[m][:, t0 : t0 + 512], ps, 512,
                                           ck_t, sk_t, t0, swps, "sw")
                        for tt in range(6):
                            v_block(tt, p1ps, "p1")

                    # scope 2: all attention + P4, one psum scope. The
                    # leftover projections (k tb2, q tb1, v tt6-9) borrow the
                    # yps ring (P4 only needs it at the very end) and act as
                    # PE filler under the ACT-paced attention pipeline.
                    with (
                        tc.tile_pool(name="wop", bufs=1) as wop,
                        tc.tile_pool(name="sps", bufs=3, space="PSUM") as sps,
                        tc.tile_pool(name="avps", bufs=3, space="PSUM") as avps,
                        tc.tile_pool(name="yps", bufs=2, space="PSUM") as yps,
                    ):
                        wos = [wop.tile([128, D], BF16, name=f"wos{i}", tag=f"wos{i}") for i in range(8)]
                        for kt in range(8):
                            nc.sync.dma_start(wos[kt][:], wo[kt * 128 : (kt + 1) * 128, :])
                        for h in range(H):
                            attn_head(0, h, sps, avps)
                        for tt in range(6, 10):
                            v_block(tt, yps, "y")
                        for m in range(8):
                            ps = yps.tile([128, 512], F32, name="y", tag="y")
                            proj_block(ps, 256, wks, m * 128, 1024)
                            rope_block(kT[m][:, 1024:1280], ps, 256,
                                       ck_t, sk_t, 1024, yps, "y")
                            ps = yps.tile([128, 512], F32, name="y", tag="y")
                            proj_block(ps, 512, wqs, m * 128, WINDOW + 512)
                            rope_block(qT[m][:, 512:1024], ps, 512,
                                       cq_t, sq_t, 512, yps, "y")
                        for h in range(H):
                            attn_head(1, h, sps, avps)
                        for qb in range(NQB):
                            for mo in range(8):
                                if qb == 1:
                                    # heads are done; borrow their idle rings
                                    pool, tag = [(yps, "y"), (sps, "s"), (avps, "av")][mo % 3]
                                else:
                                    pool, tag = yps, "y"
                                ps = pool.tile([128, QB], F32, name="y", tag=tag)
                                for kf in range(8):
                                    nc.tensor.matmul(
                                        ps[:],
                                        wos[kf][:, mo * 128 : (mo + 1) * 128],
                                        aT[:, kf * TQ + qb * QB : kf * TQ + (qb + 1) * QB],
                                        start=(kf == 0),
                                        stop=(kf == 7),
                                    )
                                ys = yst.tile([128, QB], F32, name="ys", tag="ys")
                                nc.scalar.copy(ys[:], ps[:])
                                nc.sync.dma_start(
                                    yt[mo * 128 : (mo + 1) * 128, qb * QB : (qb + 1) * QB],
                                    ys[:],
                                )

    return nc


# ----------------------------------------------------------------------------
# Host-side shard preparation


def _rope_tables(pos):
    """[128, len(pos)] cos and signed-sin tables for the 2-head tile layout."""
    inv_freq = 1.0 / (ROPE_BASE ** (np.arange(0, HD, 2, dtype=np.float32) / HD))  # [32]
    freqs = np.outer(pos.astype(np.float32), inv_freq)  # [T, 32]
    c32 = np.cos(freqs).astype(np.float32).T  # [32, T]
    s32 = np.sin(freqs).astype(np.float32).T
    ctab = np.tile(c32, (4, 1))  # rows r use freq r%32
    sgn = np.repeat(np.array([-1.0, 1.0, -1.0, 1.0], dtype=np.float32), 32)
    stab = np.tile(s32, (4, 1)) * sgn[:, None]
    return np.ascontiguousarray(ctab), np.ascontiguousarray(stab)


def _perm_matrix():
    p = np.zeros((128, 128), dtype=np.float32)
    for i in range(128):
        j = i + 32 if (i // 32) % 2 == 0 else i - 32
        p[i, j] = 1.0
    return p


_BF = ml_dtypes.bfloat16


def _shared_inputs(Wqkv, Wout):
    Wqkv = np.asarray(Wqkv, dtype=np.float32)
    return {
        "wq": np.ascontiguousarray(Wqkv[:, 0:D]).astype(_BF),
        "wk": np.ascontiguousarray(Wqkv[:, D : 2 * D]).astype(_BF),
        "wv": np.ascontiguousarray(Wqkv[:, 2 * D : 3 * D]).astype(_BF),
        "wo": np.ascontiguousarray(np.asarray(Wout, dtype=np.float32)).astype(_BF),
        "perm": _perm_matrix().astype(_BF),
    }


def _core_inputs(x, shared, core):
    n, half = core // 2, core % 2
    q0 = half * TQ            # first query token (global)
    e0 = q0 - WINDOW          # first ext kv token (global, may be negative)

    x_ext = np.zeros((TE, D), dtype=np.float32)
    lo, hi = max(e0, 0), min(e0 + TE, T)
    x_ext[lo - e0 : hi - e0] = x[n, lo:hi]
    xt = np.ascontiguousarray(x_ext.T).astype(_BF)

    pos_q = np.arange(q0, q0 + TQ)
    pos_k = np.clip(np.arange(e0, e0 + TE), 0, T - 1)
    cqt, sqt = _rope_tables(pos_q)
    ckt, skt = _rope_tables(pos_k)

    # mask [128 kt, NQB*3*QB] in the packed 3-bank layout
    mask = np.zeros((128, NQB * 3 * QB), dtype=np.float32)
    for qb in range(NQB):
        for bank, bcol, kb, qoff, w in SC_PACK:
            jj = e0 + qb * QB + kb * KB + np.arange(KB)       # global key index
            ii = q0 + qb * QB + qoff + np.arange(w)           # global query index
            valid = (
                (np.abs(jj[:, None] - ii[None, :]) <= WINDOW)
                & (jj[:, None] >= 0)
                & (jj[:, None] < T)
            )
            c0 = (qb * 3 + bank) * QB + bcol
            mask[:, c0 : c0 + w] = valid
    out = dict(shared)
    out.update({
        "xt": xt,
        "cq": cqt.astype(_BF),
        "sq": sqt,
        "ck": ckt.astype(_BF),
        "sk": skt,
        "mask": mask.astype(_BF),
    })
    return out


_NC_CACHE = {}


def _get_nc():
    if "nc" not in _NC_CACHE:
        _NC_CACHE["nc"] = build_nc()
    return _NC_CACHE["nc"]


def kernel(x, Wqkv, Wout, bout, _trace=False, _trace_kwargs=None):
    x = np.asarray(x, dtype=np.float32)
    shared = _shared_inputs(Wqkv, Wout)
    in_maps = [_core_inputs(x, shared, c) for c in range(NCORES)]
    nc = _get_nc()
    kw = {}
    if _trace:
        kw = {"trace": True, "trace_kwargs": _trace_kwargs or {}}
    res = run_bass_kernel_spmd(nc, in_maps, core_ids=list(range(NCORES)), **kw)
    out = np.empty((N, T, D), dtype=np.float32)
    for c in range(NCORES):
        n, half = c // 2, c % 2
        out[n, half * TQ : (half + 1) * TQ] = res.results[c]["yt"].T
    out += np.asarray(bout, dtype=np.float32)[None, None, :]
    kernel._last_results = res
    return out


# revision 29
# speedup vs baseline: 1.1110x; 1.0406x over previous
"""Sliding-window multi-head attention (N=4, T=2048, D=1024, H=16, hd=64,
rotary over all 64 dims, window (128,128)) on 8 Trainium2 NeuronCores.

Sharding: data-parallel over (batch, sequence-half): core c handles batch
c//2, query tokens [h*1024, (h+1)*1024) with a 128-token KV halo on each
side (zero-padded at sequence edges, masked in softmax).

v2 design (vs v1 baseline at 338us):
  - all matmul operands bf16 (halves DMA bytes; kills the fp32r 4x
    small-free-dim penalty on banded score matmuls)
  - consolidated DMAs: one per 128-row chunk of each weight (~45 total vs
    265) so the serial HWDGE resource stops gating P1/P4
  - P1 q-projection in kt-outer waves so PE starts as soon as the first
    (x, w) chunk pair lands instead of after the full weight load
  - scores packed 3 psum banks per (qb, head) -> 3 exps instead of 6
  - AV accumulation uses per-region start flags (no zeroing matmul)
  - softmax denominators: ones-column rides in V; reciprocal row is
    broadcast across partitions with a stride-0 SBUF->SBUF DMA (no PE
    broadcast matmul, no ACT copy)
  - wout preloaded; output projection interleaved per query block
"""

import math

import ml_dtypes
import numpy as np

import bass_rust
import concourse.bass as bass
import concourse.mybir as mybir
import concourse.tile as tile
from concourse.bass_utils import run_bass_kernel_spmd
from concourse.vector_clock import ScopedClock

# ----------------------------------------------------------------------------
# Problem constants (hardcoded per the harness contract)
N, T, D = 4, 2048, 1024
H, HD = 16, 64
WINDOW = 128
ROPE_BASE = 10000.0
SCALE = 1.0 / math.sqrt(HD)

NCORES = 8
TQ = 1024             # query tokens per core
TE = TQ + 2 * WINDOW  # 1280 extended kv tokens per core
QB = 512              # query block
NQB = TQ // QB        # 2
KB = 128              # key block
NKB = (QB + 2 * WINDOW) // KB  # 6 key blocks per query block

VS = HD + 1   # per-head column stride in V (col 64 = ones)
VW = H * VS   # 1040

F32 = mybir.dt.float32
F32R = mybir.dt.float32r
BF16 = mybir.dt.bfloat16

# scores bank packing: (bank, bank_col, kb, q_off, width)
SC_PACK = [
    (0, 0,   0, 0,   128),
    (0, 128, 1, 0,   256),
    (0, 384, 2, 0,   128),
    (1, 0,   2, 128, 256),
    (1, 256, 3, 128, 256),
    (2, 0,   3, 384, 128),
    (2, 128, 4, 256, 256),
    (2, 384, 5, 384, 128),
]

# AV sub-matmuls: (bank, bank_col, kb, q_off, width)
# start=True ONLY on the first: it marks the whole 2KB psum bank pending-zero
# (ZERO_REGION_SIZE); later matmuls overwrite-then-accumulate per column via
# the per-element has_written bits. Ordered so every matmul's touched region
# is uniformly pending (first column write) or uniformly written (accumulate).
AV_PACK = [
    (0, 0,   0, 0,   128),
    (0, 128, 1, 0,   128),
    (0, 256, 1, 128, 128),
    (0, 384, 2, 0,   128),
    (1, 0,   2, 128, 128),
    (1, 128, 2, 256, 128),
    (1, 256, 3, 128, 256),
    (2, 0,   3, 384, 128),
    (2, 128, 4, 256, 256),
    (2, 384, 5, 384, 128),
]

_MAXW = 1  # this container's walrus accepts one sync wait per instruction


class SplitWaitTC(tile.TileContext):
    """TileContext that spreads multi-sem waits over NoOp carriers so every
    instruction carries at most one sync wait (codegen limit here)."""

    _waitnop_counter = 0

    def _split_waits(self, inst, commit):
        si = getattr(inst, "sync_info", None)
        if si is None:
            return
        waits = list(si.on_wait)
        if len(waits) <= _MAXW:
            return
        ups = list(si.on_update)
        head, keep = waits[:-_MAXW], waits[-_MAXW:]
        for w in head:
            nop = bass_rust.InstNoOp()
            nop.engine = inst.engine
            SplitWaitTC._waitnop_counter += 1
            nop.name = f"I-waitnop-{SplitWaitTC._waitnop_counter}"
            nop.bass_nofuse = True
            nop.sync_info = bass_rust.SyncInfo(on_wait=[w], on_update=[])
            commit(nop)
        inst.sync_info = bass_rust.SyncInfo(on_wait=keep, on_update=ups)

    def _commit_and_lower(self, inst, original_block, old_bb_map, bb_to_exit_bb):
        if isinstance(inst, mybir.Instruction) and not isinstance(
            inst, (tile.BassTileRelease,)
        ):
            self._split_waits(
                inst,
                lambda nop: super(SplitWaitTC, self)._commit_and_lower(
                    nop, original_block, old_bb_map, bb_to_exit_bb
                ),
            )
        return super()._commit_and_lower(inst, original_block, old_bb_map, bb_to_exit_bb)

    def _drain_and_barrier(self, tick_clock, wait_clock):
        probe = self.nc.sync.nop(nofuse=True)
        wait_clock.add_sem_waits(probe.ins, ScopedClock({None: tick_clock.global_clock}))
        si = probe.ins.sync_info
        waits = list(si.on_wait) if si is not None else []
        ups = list(si.on_update) if si is not None else []
        if len(waits) > _MAXW:
            probe.ins.sync_info = bass_rust.SyncInfo(on_wait=waits[:_MAXW], on_update=ups)
            rest = waits[_MAXW:]
            while rest:
                chunk, rest = rest[:_MAXW], rest[_MAXW:]
                n = self.nc.sync.nop(nofuse=True)
                n.ins.sync_info = bass_rust.SyncInfo(on_wait=chunk, on_update=[])
        self.nc.sync.drain()
        self.nc.all_engine_barrier()
        assert self.sems is not None
        popped = self.nc._tile_sem_poison_stack.pop()
        assert popped is self._sem_poison
        self.nc.clear_and_free_semaphores(list(self.sems.allocated().values()))
        self.nc.all_engine_barrier()


def _bcast_row(ap, n):
    """View of a single-partition row AP replicated n times via a stride-0
    free dim (the partition dim itself must keep a nonzero step)."""
    dims = [list(d) for d in list(ap.ap)]
    return bass.AP(ap.tensor, ap.offset, [dims[0], [0, n]] + dims[1:])


# ----------------------------------------------------------------------------
# Device program


def build_nc():
    nc = bass.Bass("TRN2", target_bir_lowering=False, debug=False, num_devices=NCORES)

    xt = nc.declare_dram_parameter("xt", [D, TE], BF16, isOutput=False)
    wq = nc.declare_dram_parameter("wq", [D, D], BF16, isOutput=False)
    wk = nc.declare_dram_parameter("wk", [D, D], BF16, isOutput=False)
    wv = nc.declare_dram_parameter("wv", [D, D], BF16, isOutput=False)
    wo = nc.declare_dram_parameter("wo", [D, D], BF16, isOutput=False)
    cq = nc.declare_dram_parameter("cq", [128, TQ], BF16, isOutput=False)
    sq = nc.declare_dram_parameter("sq", [128, TQ], F32, isOutput=False)
    ck = nc.declare_dram_parameter("ck", [128, TE], BF16, isOutput=False)
    sk = nc.declare_dram_parameter("sk", [128, TE], F32, isOutput=False)
    maskd = nc.declare_dram_parameter("mask", [128, NQB * 3 * QB], BF16, isOutput=False)
    permd = nc.declare_dram_parameter("perm", [128, 128], BF16, isOutput=False)
    yt = nc.declare_dram_parameter("yt", [D, TQ], F32, isOutput=True)

    AF = mybir.ActivationFunctionType

    with nc.allow_low_precision(reason="bf16 matmuls; fp32 accumulate"), SplitWaitTC(nc) as tc:
        with (
            tc.tile_pool(name="const", bufs=1) as constp,
            tc.tile_pool(name="persist", bufs=1) as persist,
        ):
            perm_t = constp.tile([128, 128], BF16, name="perm", tag="perm")
            nc.sync.dma_start(perm_t[:], permd[:])

            # persistent activations
            qT = [persist.tile([128, TQ], BF16, name=f"qT{i}", tag=f"qT{i}") for i in range(8)]
            kT = [persist.tile([128, TE], BF16, name=f"kT{i}", tag=f"kT{i}") for i in range(8)]
            vp = [persist.tile([128, VW], BF16, name=f"vp{i}", tag=f"vp{i}") for i in range(10)]
            aT = persist.tile([128, 8 * TQ], BF16, name="aT", tag="aT")

            # shared attention-side SBUF pools (span all phases)
            with (
                tc.tile_pool(name="maskp", bufs=1) as maskp,
                tc.tile_pool(name="probs", bufs=7) as probsp,
                tc.tile_pool(name="rcp", bufs=3) as rcp,
                tc.tile_pool(name="bcp", bufs=3) as bcp,
                tc.tile_pool(name="anp", bufs=2) as anp,
                tc.tile_pool(name="yst", bufs=2) as yst,
            ):
                mask_t = maskp.tile([128, NQB * 3 * QB], BF16, name="mask", tag="mask")

                def attn_head(qb, h, spool, apool):
                    ft = h // 2
                    p0 = (h % 2) * 64
                    banks = [
                        spool.tile([128, QB], F32, name="s", tag="s") for _ in range(3)
                    ]
                    for bank, bcol, kb, qoff, w in SC_PACK:
                        kv0 = (qb * 4 + kb) * 128
                        nc.tensor.matmul(
                            banks[bank][:, bcol : bcol + w],
                            kT[ft][p0 : p0 + 64, kv0 : kv0 + KB],
                            qT[ft][p0 : p0 + 64, qb * QB + qoff : qb * QB + qoff + w],
                            start=True,
                            stop=True,
                        )
                    prs = []
                    for b in range(3):
                        pr = probsp.tile([128, QB], BF16, name="pr", tag="pr")
                        nc.scalar.activation(pr[:], banks[b][:], AF.Exp, scale=SCALE)
                        mcol = (qb * 3 + b) * QB
                        if b == 2 and h % 2 == 1:
                            # offload 0.5 of three mask muls to Pool
                            nc.gpsimd.tensor_mul(pr[:], pr[:], mask_t[:, mcol : mcol + QB])
                        else:
                            nc.vector.tensor_mul(pr[:], pr[:], mask_t[:, mcol : mcol + QB])
                        prs.append(pr)
                    # AV: start only on the first matmul (marks whole bank
                    # pending-zero); later ones overwrite-then-accumulate per
                    # column. The V ones column lands in output row 64 =
                    # masked prob sums.
                    psX = apool.tile([128, QB], F32, name="av", tag="av")
                    nav = len(AV_PACK)
                    for i, (bank, bcol, kb, qoff, w) in enumerate(AV_PACK):
                        vt = qb * 4 + kb
                        nc.tensor.matmul(
                            psX[: VS, qoff : qoff + w],
                            vp[vt][:, h * VS : h * VS + VS],
                            prs[bank][:, bcol : bcol + w],
                            start=(i == 0),
                            stop=(i == nav - 1),
                            skip_group_check=True,
                        )
                    # normalize: reciprocal row -> stride-0 DMA broadcast
                    rc = rcp.tile([VS, QB], F32, name="rc", tag="rc")
                    nc.vector.reciprocal(rc[HD : HD + 1, :], psX[HD : HD + 1, :])
                    bc = bcp.tile([64, QB], F32, name="bc", tag="bc")
                    nc.sync.dma_start(bc[:, :], _bcast_row(rc[HD : HD + 1, :], 64))
                    if p0 == 0:
                        nc.vector.tensor_mul(
                            aT[0:HD, ft * TQ + qb * QB : ft * TQ + (qb + 1) * QB],
                            psX[0:HD, :],
                            bc[:],
                        )
                    else:
                        an = anp.tile([64, QB], BF16, name="an", tag="an")
                        nc.vector.tensor_mul(an[:], psX[0:HD, :], bc[:])
                        # partition-shift DMA into the odd-head rows of aT
                        nc.sync.dma_start(
                            aT[HD:128, ft * TQ + qb * QB : ft * TQ + (qb + 1) * QB],
                            an[:],
                        )

                # ---------------- P1 + attention qb0 -------------------------
                with (
                    tc.tile_pool(name="p1data", bufs=1) as p1data,
                    tc.tile_pool(name="stage", bufs=2) as stage,
                    tc.tile_pool(name="ropetmp", bufs=2) as ropetmp,
                ):
                    xs = [p1data.tile([128, TE], BF16, name=f"xs{i}", tag=f"xs{i}") for i in range(8)]
                    wqs = [p1data.tile([128, D], BF16, name=f"wqs{i}", tag=f"wqs{i}") for i in range(8)]
                    wks = [p1data.tile([128, D], BF16, name=f"wks{i}", tag=f"wks{i}") for i in range(8)]
                    wvs = [p1data.tile([128, D], BF16, name=f"wvs{i}", tag=f"wvs{i}") for i in range(8)]
                    # DMA order = issue order on SP: q weights + x first.
                    for kt in range(8):
                        nc.sync.dma_start(wqs[kt][:], wq[kt * 128 : (kt + 1) * 128, :])
                        nc.sync.dma_start(xs[kt][:], xt[kt * 128 : (kt + 1) * 128, :])
                    cq_t = p1data.tile([128, TQ], BF16, name="cq", tag="cq")
                    nc.sync.dma_start(cq_t[:], cq[:])
                    sq_t = p1data.tile([128, TQ], F32, name="sq", tag="sq")
                    nc.sync.dma_start(sq_t[:], sq[:])
                    for kt in range(8):
                        nc.sync.dma_start(wks[kt][:], wk[kt * 128 : (kt + 1) * 128, :])
                    ck_t = p1data.tile([128, TE], BF16, name="ck", tag="ck")
                    nc.sync.dma_start(ck_t[:], ck[:])
                    sk_t = p1data.tile([128, TE], F32, name="sk", tag="sk")
                    nc.sync.dma_start(sk_t[:], sk[:])
                    for kt in range(8):
                        nc.sync.dma_start(wvs[kt][:], wv[kt * 128 : (kt + 1) * 128, :])
                    nc.sync.dma_start(mask_t[:], maskd[:])

                    def rope_block(dest_slice, ps, nt, ctab, stab, c0, swpool, swtag):
                        raw = stage.tile([128, 512], BF16, name="raw", tag="raw")
                        nc.scalar.copy(raw[:, :nt], ps[:, :nt])
                        psw = swpool.tile([128, 512], F32, name="sw", tag=swtag)
                        nc.tensor.matmul(
                            psw[:, :nt], perm_t[:], raw[:, :nt], start=True, stop=True
                        )
                        t1 = ropetmp.tile([128, 512], BF16, name="t1", tag="t1")
                        nc.gpsimd.tensor_mul(t1[:, :nt], raw[:, :nt], ctab[:, c0 : c0 + nt])
                        t2 = ropetmp.tile([128, 512], BF16, name="t2", tag="t2")
                        nc.vector.tensor_mul(t2[:, :nt], psw[:, :nt], stab[:, c0 : c0 + nt])
                        nc.vector.tensor_add(dest_slice, t1[:, :nt], t2[:, :nt])

                    def proj_block(ps, nt, wtiles, mcol, xcol):
                        for kt in range(8):
                            nc.tensor.matmul(
                                ps[:, :nt],
                                wtiles[kt][:, mcol : mcol + 128],
                                xs[kt][:, xcol : xcol + nt],
                                start=(kt == 0),
                                stop=(kt == 7),
                            )

                    def v_block(tt, pool, tag):
                        for fb in range(2):
                            ps = pool.tile([128, 512], F32, name="p1", tag=tag)
                            for kt in range(8):
                                nc.tensor.matmul(
                                    ps[:],
                                    xs[kt][:, tt * 128 : (tt + 1) * 128],
                                    wvs[kt][:, fb * 512 : (fb + 1) * 512],
                                    start=(kt == 0),
                                    stop=(kt == 7),
                                )
                            dst = vp[tt][:].rearrange("p (h s) -> p h s", s=VS)[
                                :, fb * 8 : (fb + 1) * 8, :HD
                            ]
                            nc.vector.tensor_copy(dst, ps[:].rearrange("p (h s) -> p h s", s=HD))
                        onescols = vp[tt][:].rearrange("p (h s) -> p h s", s=VS)[:, :, HD:]
                        nc.gpsimd.memset(onescols, 1.0)

                    # scope 1: q tb0 (kt-outer waves for early PE start),
                    # k tb0/tb1, v tt0..5 — everything attention qb0 needs
                    with (
                        tc.tile_pool(name="p1ps", bufs=6, space="PSUM") as p1ps,
                        tc.tile_pool(name="swps", bufs=2, space="PSUM") as swps,
                    ):
                        for wave in (list(range(6)), list(range(6, 8))):
                            pss = {}
                            for m in wave:
                                pss[m] = p1ps.tile([128, 512], F32, name="p1", tag="p1")
                            for kt in range(8):
                                for m in wave:
                                    nc.tensor.matmul(
                                        pss[m][:],
                                        wqs[kt][:, m * 128 : (m + 1) * 128],
                                        xs[kt][:, WINDOW : WINDOW + 512],
                                        start=(kt == 0),
                                        stop=(kt == 7),
                                    )
                            for m in wave:
                                rope_block(qT[m][:, 0:512], pss[m], 512, cq_t, sq_t, 0,
                                           swps, "sw")
                        for m in range(8):
                            for tb in range(2):
                                t0 = tb * 512
                                ps = p1ps.tile([128, 512], F32, name="p1", tag="p1")
                                proj_block(ps, 512, wks, m * 128, t0)
                                rope_block(kT[m][:, t0 : t0 + 512], ps, 512,
                                           ck_t, sk_t, t0, swps, "sw")
                        for tt in range(6):
                            v_block(tt, p1ps, "p1")

                    # scope 2: all attention + P4, one psum scope. The
                    # leftover projections (k tb2, q tb1, v tt6-9) borrow the
                    # yps ring (P4 only needs it at the very end) and act as
                    # PE filler under the ACT-paced attention pipeline.
                    with (
                        tc.tile_pool(name="wop", bufs=1) as wop,
                        tc.tile_pool(name="sps", bufs=3, space="PSUM") as sps,
                        tc.tile_pool(name="avps", bufs=3, space="PSUM") as avps,
                        tc.tile_pool(name="yps", bufs=2, space="PSUM") as yps,
                    ):
                        wos = [wop.tile([128, D], BF16, name=f"wos{i}", tag=f"wos{i}") for i in range(8)]
                        for kt in range(8):
                            nc.sync.dma_start(wos[kt][:], wo[kt * 128 : (kt + 1) * 128, :])
                        def leftover(i):
                            # PE filler: v tt6-9 first (qb1 AV needs them),
                            # then per-m k tb2 + q tb1
                            if i < 4:
                                v_block(6 + i, yps, "y")
                                return
                            m = i - 4
                            ps = yps.tile([128, 512], F32, name="y", tag="y")
                            proj_block(ps, 256, wks, m * 128, 1024)
                            rope_block(kT[m][:, 1024:1280], ps, 256,
                                       ck_t, sk_t, 1024, yps, "y")
                            ps = yps.tile([128, 512], F32, name="y", tag="y")
                            proj_block(ps, 512, wqs, m * 128, WINDOW + 512)
                            rope_block(qT[m][:, 512:1024], ps, 512,
                                       cq_t, sq_t, 512, yps, "y")

                        li = 0
                        for h in range(H):
                            attn_head(0, h, sps, avps)
                            while li < 12 and li <= (h * 12) // H:
                                leftover(li)
                                li += 1
                        while li < 12:
                            leftover(li)
                            li += 1
                        for h in range(H):
                            attn_head(1, h, sps, avps)
                        for qb in range(NQB):
                            for mo in range(8):
                                if qb == 1:
                                    # heads are done; borrow their idle rings
                                    pool, tag = [(yps, "y"), (sps, "s"), (avps, "av")][mo % 3]
                                else:
                                    pool, tag = yps, "y"
                                ps = pool.tile([128, QB], F32, name="y", tag=tag)
                                for kf in range(8):
                                    nc.tensor.matmul(
                                        ps[:],
                                        wos[kf][:, mo * 128 : (mo + 1) * 128],
                                        aT[:, kf * TQ + qb * QB : kf * TQ + (qb + 1) * QB],
                                        start=(kf == 0),
                                        stop=(kf == 7),
                                    )
                                ys = yst.tile([128, QB], F32, name="ys", tag="ys")
                                nc.vector.tensor_copy(ys[:], ps[:])
                                nc.sync.dma_start(
                                    yt[mo * 128 : (mo + 1) * 128, qb * QB : (qb + 1) * QB],
                                    ys[:],
                                )

    return nc


# ----------------------------------------------------------------------------
# Host-side shard preparation


def _rope_tables(pos):
    """[128, len(pos)] cos and signed-sin tables for the 2-head tile layout."""
    inv_freq = 1.0 / (ROPE_BASE ** (np.arange(0, HD, 2, dtype=np.float32) / HD))  # [32]
    freqs = np.outer(pos.astype(np.float32), inv_freq)  # [T, 32]
    c32 = np.cos(freqs).astype(np.float32).T  # [32, T]
    s32 = np.sin(freqs).astype(np.float32).T
    ctab = np.tile(c32, (4, 1))  # rows r use freq r%32
    sgn = np.repeat(np.array([-1.0, 1.0, -1.0, 1.0], dtype=np.float32), 32)
    stab = np.tile(s32, (4, 1)) * sgn[:, None]
    return np.ascontiguousarray(ctab), np.ascontiguousarray(stab)


def _perm_matrix():
    p = np.zeros((128, 128), dtype=np.float32)
    for i in range(128):
        j = i + 32 if (i // 32) % 2 == 0 else i - 32
        p[i, j] = 1.0
    return p


_BF = ml_dtypes.bfloat16


def _shared_inputs(Wqkv, Wout):
    Wqkv = np.asarray(Wqkv, dtype=np.float32)
    return {
        "wq": np.ascontiguousarray(Wqkv[:, 0:D]).astype(_BF),
        "wk": np.ascontiguousarray(Wqkv[:, D : 2 * D]).astype(_BF),
        "wv": np.ascontiguousarray(Wqkv[:, 2 * D : 3 * D]).astype(_BF),
        "wo": np.ascontiguousarray(np.asarray(Wout, dtype=np.float32)).astype(_BF),
        "perm": _perm_matrix().astype(_BF),
    }


def _core_inputs(x, shared, core):
    n, half = core // 2, core % 2
    q0 = half * TQ            # first query token (global)
    e0 = q0 - WINDOW          # first ext kv token (global, may be negative)

    x_ext = np.zeros((TE, D), dtype=np.float32)
    lo, hi = max(e0, 0), min(e0 + TE, T)
    x_ext[lo - e0 : hi - e0] = x[n, lo:hi]
    xt = np.ascontiguousarray(x_ext.T).astype(_BF)

    pos_q = np.arange(q0, q0 + TQ)
    pos_k = np.clip(np.arange(e0, e0 + TE), 0, T - 1)
    cqt, sqt = _rope_tables(pos_q)
    ckt, skt = _rope_tables(pos_k)

    # mask [128 kt, NQB*3*QB] in the packed 3-bank layout
    mask = np.zeros((128, NQB * 3 * QB), dtype=np.float32)
    for qb in range(NQB):
        for bank, bcol, kb, qoff, w in SC_PACK:
            jj = e0 + qb * QB + kb * KB + np.arange(KB)       # global key index
            ii = q0 + qb * QB + qoff + np.arange(w)           # global query index
            valid = (
                (np.abs(jj[:, None] - ii[None, :]) <= WINDOW)
                & (jj[:, None] >= 0)
                & (jj[:, None] < T)
            )
            c0 = (qb * 3 + bank) * QB + bcol
            mask[:, c0 : c0 + w] = valid
    out = dict(shared)
    out.update({
        "xt": xt,
        "cq": cqt.astype(_BF),
        "sq": sqt,
        "ck": ckt.astype(_BF),
        "sk": skt,
        "mask": mask.astype(_BF),
    })
    return out


_NC_CACHE = {}


def _get_nc():
    if "nc" not in _NC_CACHE:
        _NC_CACHE["nc"] = build_nc()
    return _NC_CACHE["nc"]


def kernel(x, Wqkv, Wout, bout, _trace=False, _trace_kwargs=None):
    x = np.asarray(x, dtype=np.float32)
    shared = _shared_inputs(Wqkv, Wout)
    in_maps = [_core_inputs(x, shared, c) for c in range(NCORES)]
    nc = _get_nc()
    kw = {}
    if _trace:
        kw = {"trace": True, "trace_kwargs": _trace_kwargs or {}}
    res = run_bass_kernel_spmd(nc, in_maps, core_ids=list(range(NCORES)), **kw)
    out = np.empty((N, T, D), dtype=np.float32)
    for c in range(NCORES):
        n, half = c // 2, c % 2
        out[n, half * TQ : (half + 1) * TQ] = res.results[c]["yt"].T
    out += np.asarray(bout, dtype=np.float32)[None, None, :]
    kernel._last_results = res
    return out


# revision 37
# speedup vs baseline: 1.1341x; 1.0208x over previous
"""Sliding-window multi-head attention (N=4, T=2048, D=1024, H=16, hd=64,
rotary over all 64 dims, window (128,128)) on 8 Trainium2 NeuronCores.

Sharding: data-parallel over (batch, sequence-half): core c handles batch
c//2, query tokens [h*1024, (h+1)*1024) with a 128-token KV halo on each
side (zero-padded at sequence edges, masked in softmax).

v2 design (vs v1 baseline at 338us):
  - all matmul operands bf16 (halves DMA bytes; kills the fp32r 4x
    small-free-dim penalty on banded score matmuls)
  - consolidated DMAs: one per 128-row chunk of each weight (~45 total vs
    265) so the serial HWDGE resource stops gating P1/P4
  - P1 q-projection in kt-outer waves so PE starts as soon as the first
    (x, w) chunk pair lands instead of after the full weight load
  - scores packed 3 psum banks per (qb, head) -> 3 exps instead of 6
  - AV accumulation uses per-region start flags (no zeroing matmul)
  - softmax denominators: ones-column rides in V; reciprocal row is
    broadcast across partitions with a stride-0 SBUF->SBUF DMA (no PE
    broadcast matmul, no ACT copy)
  - wout preloaded; output projection interleaved per query block
"""

import math

import ml_dtypes
import numpy as np

import bass_rust
import concourse.bass as bass
import concourse.mybir as mybir
import concourse.tile as tile
from concourse.bass_utils import run_bass_kernel_spmd
from concourse.vector_clock import ScopedClock

# ----------------------------------------------------------------------------
# Problem constants (hardcoded per the harness contract)
N, T, D = 4, 2048, 1024
H, HD = 16, 64
WINDOW = 128
ROPE_BASE = 10000.0
SCALE = 1.0 / math.sqrt(HD)

NCORES = 8
TQ = 1024             # query tokens per core
TE = TQ + 2 * WINDOW  # 1280 extended kv tokens per core
QB = 512              # query block
NQB = TQ // QB        # 2
KB = 128              # key block
NKB = (QB + 2 * WINDOW) // KB  # 6 key blocks per query block

VS = HD + 1   # per-head column stride in V (col 64 = ones)
VW = H * VS   # 1040

F32 = mybir.dt.float32
F32R = mybir.dt.float32r
BF16 = mybir.dt.bfloat16

# scores bank packing: (bank, bank_col, kb, q_off, width)
SC_PACK = [
    (0, 0,   0, 0,   128),
    (0, 128, 1, 0,   256),
    (0, 384, 2, 0,   128),
    (1, 0,   2, 128, 256),
    (1, 256, 3, 128, 256),
    (2, 0,   3, 384, 128),
    (2, 128, 4, 256, 256),
    (2, 384, 5, 384, 128),
]

# AV sub-matmuls: (bank, bank_col, kb, q_off, width)
# start=True ONLY on the first: it marks the whole 2KB psum bank pending-zero
# (ZERO_REGION_SIZE); later matmuls overwrite-then-accumulate per column via
# the per-element has_written bits. Ordered so every matmul's touched region
# is uniformly pending (first column write) or uniformly written (accumulate).
AV_PACK = [
    (0, 0,   0, 0,   128),
    (0, 128, 1, 0,   128),
    (0, 256, 1, 128, 128),
    (0, 384, 2, 0,   128),
    (1, 0,   2, 128, 128),
    (1, 128, 2, 256, 128),
    (1, 256, 3, 128, 256),
    (2, 0,   3, 384, 128),
    (2, 128, 4, 256, 256),
    (2, 384, 5, 384, 128),
]

_MAXW = 1  # this container's walrus accepts one sync wait per instruction


class SplitWaitTC(tile.TileContext):
    """TileContext that spreads multi-sem waits over NoOp carriers so every
    instruction carries at most one sync wait (codegen limit here)."""

    _waitnop_counter = 0

    def _split_waits(self, inst, commit):
        si = getattr(inst, "sync_info", None)
        if si is None:
            return
        waits = list(si.on_wait)
        if len(waits) <= _MAXW:
            return
        ups = list(si.on_update)
        head, keep = waits[:-_MAXW], waits[-_MAXW:]
        for w in head:
            nop = bass_rust.InstNoOp()
            nop.engine = inst.engine
            SplitWaitTC._waitnop_counter += 1
            nop.name = f"I-waitnop-{SplitWaitTC._waitnop_counter}"
            nop.bass_nofuse = True
            nop.sync_info = bass_rust.SyncInfo(on_wait=[w], on_update=[])
            commit(nop)
        inst.sync_info = bass_rust.SyncInfo(on_wait=keep, on_update=ups)

    def _commit_and_lower(self, inst, original_block, old_bb_map, bb_to_exit_bb):
        if isinstance(inst, mybir.Instruction) and not isinstance(
            inst, (tile.BassTileRelease,)
        ):
            self._split_waits(
                inst,
                lambda nop: super(SplitWaitTC, self)._commit_and_lower(
                    nop, original_block, old_bb_map, bb_to_exit_bb
                ),
            )
        return super()._commit_and_lower(inst, original_block, old_bb_map, bb_to_exit_bb)

    def _drain_and_barrier(self, tick_clock, wait_clock):
        probe = self.nc.sync.nop(nofuse=True)
        wait_clock.add_sem_waits(probe.ins, ScopedClock({None: tick_clock.global_clock}))
        si = probe.ins.sync_info
        waits = list(si.on_wait) if si is not None else []
        ups = list(si.on_update) if si is not None else []
        if len(waits) > _MAXW:
            probe.ins.sync_info = bass_rust.SyncInfo(on_wait=waits[:_MAXW], on_update=ups)
            rest = waits[_MAXW:]
            while rest:
                chunk, rest = rest[:_MAXW], rest[_MAXW:]
                n = self.nc.sync.nop(nofuse=True)
                n.ins.sync_info = bass_rust.SyncInfo(on_wait=chunk, on_update=[])
        self.nc.sync.drain()
        self.nc.all_engine_barrier()
        assert self.sems is not None
        popped = self.nc._tile_sem_poison_stack.pop()
        assert popped is self._sem_poison
        self.nc.clear_and_free_semaphores(list(self.sems.allocated().values()))
        self.nc.all_engine_barrier()


def _bcast_row(ap, n):
    """View of a single-partition row AP replicated n times via a stride-0
    free dim (the partition dim itself must keep a nonzero step)."""
    dims = [list(d) for d in list(ap.ap)]
    return bass.AP(ap.tensor, ap.offset, [dims[0], [0, n]] + dims[1:])


# ----------------------------------------------------------------------------
# Device program


def build_nc():
    nc = bass.Bass("TRN2", target_bir_lowering=False, debug=False, num_devices=NCORES)

    xt = nc.declare_dram_parameter("xt", [D, TE], BF16, isOutput=False)
    wq = nc.declare_dram_parameter("wq", [D, D], BF16, isOutput=False)
    wk = nc.declare_dram_parameter("wk", [D, D], BF16, isOutput=False)
    wv = nc.declare_dram_parameter("wv", [D, D], BF16, isOutput=False)
    wo = nc.declare_dram_parameter("wo", [D, D], BF16, isOutput=False)
    cq = nc.declare_dram_parameter("cq", [128, TQ], BF16, isOutput=False)
    sq = nc.declare_dram_parameter("sq", [128, TQ], F32, isOutput=False)
    ck = nc.declare_dram_parameter("ck", [128, TE], BF16, isOutput=False)
    sk = nc.declare_dram_parameter("sk", [128, TE], F32, isOutput=False)
    maskd = nc.declare_dram_parameter("mask", [128, NQB * 3 * QB], BF16, isOutput=False)
    permd = nc.declare_dram_parameter("perm", [128, 128], BF16, isOutput=False)
    yt = nc.declare_dram_parameter("yt", [D, TQ], F32, isOutput=True)

    AF = mybir.ActivationFunctionType

    with nc.allow_low_precision(reason="bf16 matmuls; fp32 accumulate"), SplitWaitTC(nc) as tc:
        with (
            tc.tile_pool(name="const", bufs=1) as constp,
            tc.tile_pool(name="persist", bufs=1) as persist,
        ):
            perm_t = constp.tile([128, 128], BF16, name="perm", tag="perm")
            nc.sync.dma_start(perm_t[:], permd[:])

            # persistent activations
            qT = [persist.tile([128, TQ], BF16, name=f"qT{i}", tag=f"qT{i}") for i in range(8)]
            kT = [persist.tile([128, TE], BF16, name=f"kT{i}", tag=f"kT{i}") for i in range(8)]
            vp = [persist.tile([128, VW], BF16, name=f"vp{i}", tag=f"vp{i}") for i in range(10)]
            aT = persist.tile([128, 8 * TQ], BF16, name="aT", tag="aT")

            # shared attention-side SBUF pools (span all phases)
            with (
                tc.tile_pool(name="maskp", bufs=1) as maskp,
                tc.tile_pool(name="probs", bufs=7) as probsp,
                tc.tile_pool(name="rcp", bufs=3) as rcp,
                tc.tile_pool(name="bcp", bufs=3) as bcp,
                tc.tile_pool(name="anp", bufs=2) as anp,
                tc.tile_pool(name="yst", bufs=2) as yst,
            ):
                mask_t = maskp.tile([128, NQB * 3 * QB], BF16, name="mask", tag="mask")

                def attn_head(qb, h, spool, apool):
                    ft = h // 2
                    p0 = (h % 2) * 64
                    banks = [
                        spool.tile([128, QB], F32, name="s", tag="s") for _ in range(3)
                    ]
                    for bank, bcol, kb, qoff, w in SC_PACK:
                        kv0 = (qb * 4 + kb) * 128
                        nc.tensor.matmul(
                            banks[bank][:, bcol : bcol + w],
                            kT[ft][p0 : p0 + 64, kv0 : kv0 + KB],
                            qT[ft][p0 : p0 + 64, qb * QB + qoff : qb * QB + qoff + w],
                            start=True,
                            stop=True,
                        )
                    prs = []
                    for b in range(3):
                        pr = probsp.tile([128, QB], BF16, name="pr", tag="pr")
                        nc.scalar.activation(pr[:], banks[b][:], AF.Exp, scale=SCALE)
                        mcol = (qb * 3 + b) * QB
                        if (b == 2 and (h % 2 == 1 or qb == 1)) or (
                            b == 1 and qb == 1
                        ):
                            # offload 0.5 (qb0) / 1 (qb1) of the mask muls to
                            # Pool - DVE is the qb1-endgame pacer, Pool idles
                            nc.gpsimd.tensor_mul(pr[:], pr[:], mask_t[:, mcol : mcol + QB])
                        else:
                            nc.vector.tensor_mul(pr[:], pr[:], mask_t[:, mcol : mcol + QB])
                        prs.append(pr)
                    # AV: start only on the first matmul (marks whole bank
                    # pending-zero); later ones overwrite-then-accumulate per
                    # column. The V ones column lands in output row 64 =
                    # masked prob sums.
                    psX = apool.tile([128, QB], F32, name="av", tag="av")
                    nav = len(AV_PACK)
                    for i, (bank, bcol, kb, qoff, w) in enumerate(AV_PACK):
                        vt = qb * 4 + kb
                        nc.tensor.matmul(
                            psX[: VS, qoff : qoff + w],
                            vp[vt][:, h * VS : h * VS + VS],
                            prs[bank][:, bcol : bcol + w],
                            start=(i == 0),
                            stop=(i == nav - 1),
                            skip_group_check=True,
                        )
                    # normalize: reciprocal row -> stride-0 DMA broadcast
                    rc = rcp.tile([VS, QB], F32, name="rc", tag="rc")
                    nc.vector.reciprocal(rc[HD : HD + 1, :], psX[HD : HD + 1, :])
                    bc = bcp.tile([64, QB], F32, name="bc", tag="bc")
                    nc.sync.dma_start(bc[:, :], _bcast_row(rc[HD : HD + 1, :], 64))
                    if p0 == 0:
                        nc.vector.tensor_mul(
                            aT[0:HD, ft * TQ + qb * QB : ft * TQ + (qb + 1) * QB],
                            psX[0:HD, :],
                            bc[:],
                        )
                    else:
                        an = anp.tile([64, QB], BF16, name="an", tag="an")
                        nc.vector.tensor_mul(an[:], psX[0:HD, :], bc[:])
                        # partition-shift DMA into the odd-head rows of aT
                        nc.sync.dma_start(
                            aT[HD:128, ft * TQ + qb * QB : ft * TQ + (qb + 1) * QB],
                            an[:],
                        )

                # ---------------- P1 + attention qb0 -------------------------
                with (
                    tc.tile_pool(name="p1data", bufs=1) as p1data,
                    tc.tile_pool(name="stage", bufs=2) as stage,
                    tc.tile_pool(name="ropetmp", bufs=2) as ropetmp,
                ):
                    xs = [p1data.tile([128, TE], BF16, name=f"xs{i}", tag=f"xs{i}") for i in range(8)]
                    wqs = [p1data.tile([128, D], BF16, name=f"wqs{i}", tag=f"wqs{i}") for i in range(8)]
                    wks = [p1data.tile([128, D], BF16, name=f"wks{i}", tag=f"wks{i}") for i in range(8)]
                    wvs = [p1data.tile([128, D], BF16, name=f"wvs{i}", tag=f"wvs{i}") for i in range(8)]
                    # DMA order = issue order on SP: q weights + x first.
                    for kt in range(8):
                        nc.sync.dma_start(wqs[kt][:], wq[kt * 128 : (kt + 1) * 128, :])
                        nc.sync.dma_start(xs[kt][:], xt[kt * 128 : (kt + 1) * 128, :])
                    cq_t = p1data.tile([128, TQ], BF16, name="cq", tag="cq")
                    nc.sync.dma_start(cq_t[:], cq[:])
                    sq_t = p1data.tile([128, TQ], F32, name="sq", tag="sq")
                    nc.sync.dma_start(sq_t[:], sq[:])
                    for kt in range(8):
                        nc.sync.dma_start(wks[kt][:], wk[kt * 128 : (kt + 1) * 128, :])
                    ck_t = p1data.tile([128, TE], BF16, name="ck", tag="ck")
                    nc.sync.dma_start(ck_t[:], ck[:])
                    sk_t = p1data.tile([128, TE], F32, name="sk", tag="sk")
                    nc.sync.dma_start(sk_t[:], sk[:])
                    for kt in range(8):
                        nc.sync.dma_start(wvs[kt][:], wv[kt * 128 : (kt + 1) * 128, :])
                    nc.sync.dma_start(mask_t[:], maskd[:])

                    def rope_block(dest_slice, ps, nt, ctab, stab, c0, swpool, swtag):
                        raw = stage.tile([128, 512], BF16, name="raw", tag="raw")
                        nc.scalar.copy(raw[:, :nt], ps[:, :nt])
                        psw = swpool.tile([128, 512], F32, name="sw", tag=swtag)
                        nc.tensor.matmul(
                            psw[:, :nt], perm_t[:], raw[:, :nt], start=True, stop=True
                        )
                        t1 = ropetmp.tile([128, 512], BF16, name="t1", tag="t1")
                        nc.gpsimd.tensor_mul(t1[:, :nt], raw[:, :nt], ctab[:, c0 : c0 + nt])
                        t2 = ropetmp.tile([128, 512], BF16, name="t2", tag="t2")
                        nc.vector.tensor_mul(t2[:, :nt], psw[:, :nt], stab[:, c0 : c0 + nt])
                        nc.vector.tensor_add(dest_slice, t1[:, :nt], t2[:, :nt])

                    def proj_block(ps, nt, wtiles, mcol, xcol):
                        for kt in range(8):
                            nc.tensor.matmul(
                                ps[:, :nt],
                                wtiles[kt][:, mcol : mcol + 128],
                                xs[kt][:, xcol : xcol + nt],
                                start=(kt == 0),
                                stop=(kt == 7),
                            )

                    def v_block(tt, pool, tag):
                        for fb in range(2):
                            ps = pool.tile([128, 512], F32, name="p1", tag=tag)
                            for kt in range(8):
                                nc.tensor.matmul(
                                    ps[:],
                                    xs[kt][:, tt * 128 : (tt + 1) * 128],
                                    wvs[kt][:, fb * 512 : (fb + 1) * 512],
                                    start=(kt == 0),
                                    stop=(kt == 7),
                                )
                            dst = vp[tt][:].rearrange("p (h s) -> p h s", s=VS)[
                                :, fb * 8 : (fb + 1) * 8, :HD
                            ]
                            nc.vector.tensor_copy(dst, ps[:].rearrange("p (h s) -> p h s", s=HD))
                        onescols = vp[tt][:].rearrange("p (h s) -> p h s", s=VS)[:, :, HD:]
                        nc.gpsimd.memset(onescols, 1.0)

                    # scope 1: q tb0 (kt-outer waves for early PE start),
                    # k tb0/tb1, v tt0..5 — everything attention qb0 needs
                    with (
                        tc.tile_pool(name="p1ps", bufs=6, space="PSUM") as p1ps,
                        tc.tile_pool(name="swps", bufs=2, space="PSUM") as swps,
                    ):
                        for wave in (list(range(6)), list(range(6, 8))):
                            pss = {}
                            for m in wave:
                                pss[m] = p1ps.tile([128, 512], F32, name="p1", tag="p1")
                            for kt in range(8):
                                for m in wave:
                                    nc.tensor.matmul(
                                        pss[m][:],
                                        wqs[kt][:, m * 128 : (m + 1) * 128],
                                        xs[kt][:, WINDOW : WINDOW + 512],
                                        start=(kt == 0),
                                        stop=(kt == 7),
                                    )
                            for m in wave:
                                rope_block(qT[m][:, 0:512], pss[m], 512, cq_t, sq_t, 0,
                                           swps, "sw")
                        for m in range(8):
                            for tb in range(2):
                                t0 = tb * 512
                                ps = p1ps.tile([128, 512], F32, name="p1", tag="p1")
                                proj_block(ps, 512, wks, m * 128, t0)
                                rope_block(kT[m][:, t0 : t0 + 512], ps, 512,
                                           ck_t, sk_t, t0, swps, "sw")
                        for tt in range(6):
                            v_block(tt, p1ps, "p1")

                    # scope 2: all attention + P4, one psum scope. The
                    # leftover projections (k tb2, q tb1, v tt6-9) borrow the
                    # yps ring (P4 only needs it at the very end) and act as
                    # PE filler under the ACT-paced attention pipeline.
                    with (
                        tc.tile_pool(name="wop", bufs=1) as wop,
                        tc.tile_pool(name="sps", bufs=3, space="PSUM") as sps,
                        tc.tile_pool(name="avps", bufs=3, space="PSUM") as avps,
                        tc.tile_pool(name="yps", bufs=2, space="PSUM") as yps,
                    ):
                        wos = [wop.tile([128, D], BF16, name=f"wos{i}", tag=f"wos{i}") for i in range(8)]
                        for kt in range(8):
                            nc.sync.dma_start(wos[kt][:], wo[kt * 128 : (kt + 1) * 128, :])
                        def leftover(i):
                            # PE filler: v tt6-9 first (qb1 AV needs them),
                            # then per-m k tb2 + q tb1
                            if i < 4:
                                v_block(6 + i, yps, "y")
                                return
                            m = i - 4
                            ps = yps.tile([128, 512], F32, name="y", tag="y")
                            proj_block(ps, 256, wks, m * 128, 1024)
                            rope_block(kT[m][:, 1024:1280], ps, 256,
                                       ck_t, sk_t, 1024, yps, "y")
                            ps = yps.tile([128, 512], F32, name="y", tag="y")
                            proj_block(ps, 512, wqs, m * 128, WINDOW + 512)
                            rope_block(qT[m][:, 512:1024], ps, 512,
                                       cq_t, sq_t, 512, yps, "y")

                        li = 0
                        for h in range(H):
                            attn_head(0, h, sps, avps)
                            while li < 12 and li <= (h * 12) // H:
                                leftover(li)
                                li += 1
                        while li < 12:
                            leftover(li)
                            li += 1
                        for h in range(H):
                            attn_head(1, h, sps, avps)
                        for qb in range(NQB):
                            for mo in range(8):
                                if qb == 1:
                                    # heads are done; borrow their idle rings
                                    pool, tag = [(yps, "y"), (sps, "s"), (avps, "av")][mo % 3]
                                else:
                                    pool, tag = yps, "y"
                                ps = pool.tile([128, QB], F32, name="y", tag=tag)
                                for kf in range(8):
                                    nc.tensor.matmul(
                                        ps[:],
                                        wos[kf][:, mo * 128 : (mo + 1) * 128],
                                        aT[:, kf * TQ + qb * QB : kf * TQ + (qb + 1) * QB],
                                        start=(kf == 0),
                                        stop=(kf == 7),
                                    )
                                ys = yst.tile([128, QB], F32, name="ys", tag="ys")
                                nc.vector.tensor_copy(ys[:], ps[:])
                                nc.sync.dma_start(
                                    yt[mo * 128 : (mo + 1) * 128, qb * QB : (qb + 1) * QB],
                                    ys[:],
                                )

    return nc


# ----------------------------------------------------------------------------
# Host-side shard preparation


def _rope_tables(pos):
    """[128, len(pos)] cos and signed-sin tables for the 2-head tile layout."""
    inv_freq = 1.0 / (ROPE_BASE ** (np.arange(0, HD, 2, dtype=np.float32) / HD))  # [32]
    freqs = np.outer(pos.astype(np.float32), inv_freq)  # [T, 32]
    c32 = np.cos(freqs).astype(np.float32).T  # [32, T]
    s32 = np.sin(freqs).astype(np.float32).T
    ctab = np.tile(c32, (4, 1))  # rows r use freq r%32
    sgn = np.repeat(np.array([-1.0, 1.0, -1.0, 1.0], dtype=np.float32), 32)
    stab = np.tile(s32, (4, 1)) * sgn[:, None]
    return np.ascontiguousarray(ctab), np.ascontiguousarray(stab)


def _perm_matrix():
    p = np.zeros((128, 128), dtype=np.float32)
    for i in range(128):
        j = i + 32 if (i // 32) % 2 == 0 else i - 32
        p[i, j] = 1.0
    return p


_BF = ml_dtypes.bfloat16


def _shared_inputs(Wqkv, Wout):
    Wqkv = np.asarray(Wqkv, dtype=np.float32)
    return {
        "wq": np.ascontiguousarray(Wqkv[:, 0:D]).astype(_BF),
        "wk": np.ascontiguousarray(Wqkv[:, D : 2 * D]).astype(_BF),
        "wv": np.ascontiguousarray(Wqkv[:, 2 * D : 3 * D]).astype(_BF),
        "wo": np.ascontiguousarray(np.asarray(Wout, dtype=np.float32)).astype(_BF),
        "perm": _perm_matrix().astype(_BF),
    }


def _core_inputs(x, shared, core):
    n, half = core // 2, core % 2
    q0 = half * TQ            # first query token (global)
    e0 = q0 - WINDOW          # first ext kv token (global, may be negative)

    x_ext = np.zeros((TE, D), dtype=np.float32)
    lo, hi = max(e0, 0), min(e0 + TE, T)
    x_ext[lo - e0 : hi - e0] = x[n, lo:hi]
    xt = np.ascontiguousarray(x_ext.T).astype(_BF)

    pos_q = np.arange(q0, q0 + TQ)
    pos_k = np.clip(np.arange(e0, e0 + TE), 0, T - 1)
    cqt, sqt = _rope_tables(pos_q)
    ckt, skt = _rope_tables(pos_k)

    # mask [128 kt, NQB*3*QB] in the packed 3-bank layout
    mask = np.zeros((128, NQB * 3 * QB), dtype=np.float32)
    for qb in range(NQB):
        for bank, bcol, kb, qoff, w in SC_PACK:
            jj = e0 + qb * QB + kb * KB + np.arange(KB)       # global key index
            ii = q0 + qb * QB + qoff + np.arange(w)           # global query index
            valid = (
                (np.abs(jj[:, None] - ii[None, :]) <= WINDOW)
                & (jj[:, None] >= 0)
                & (jj[:, None] < T)
            )
            c0 = (qb * 3 + bank) * QB + bcol
            mask[:, c0 : c0 + w] = valid
    out = dict(shared)
    out.update({
        "xt": xt,
        "cq": cqt.astype(_BF),
        "sq": sqt,
        "ck": ckt.astype(_BF),
        "sk": skt,
        "mask": mask.astype(_BF),
    })
    return out


_NC_CACHE = {}


def _get_nc():
    if "nc" not in _NC_CACHE:
        _NC_CACHE["nc"] = build_nc()
    return _NC_CACHE["nc"]


def kernel(x, Wqkv, Wout, bout, _trace=False, _trace_kwargs=None):
    x = np.asarray(x, dtype=np.float32)
    shared = _shared_inputs(Wqkv, Wout)
    in_maps = [_core_inputs(x, shared, c) for c in range(NCORES)]
    nc = _get_nc()
    kw = {}
    if _trace:
        kw = {"trace": True, "trace_kwargs": _trace_kwargs or {}}
    res = run_bass_kernel_spmd(nc, in_maps, core_ids=list(range(NCORES)), **kw)
    out = np.empty((N, T, D), dtype=np.float32)
    for c in range(NCORES):
        n, half = c // 2, c % 2
        out[n, half * TQ : (half + 1) * TQ] = res.results[c]["yt"].T
    out += np.asarray(bout, dtype=np.float32)[None, None, :]
    kernel._last_results = res
    return out
